# revision 2
# baseline (speedup 1.0000x reference)
"""Trainium2 Bass kernel for linear-chain CRF NLL — chunked rank-1 scan.

v2 halved the serial depth with a forward/backward meet-in-the-middle; this
version removes the latency wall entirely. The chunk transfer matrices
P_i = prod_{t in chunk} (D_t E) are numerically rank-1 (E = exp(T) with
T ~ U[-0.1,0.1] is a small perturbation of the all-ones matrix, so a
64-step product contracts off-rank components by ~1e-50; validated to f32
roundoff in _sim_v3.py). Hence the time axis splits into K independent
chunks; the device only computes, per chunk and per sequence:

  alpha_i = forward pass over chunk i from all-ones state
  omega_i = backward pass over chunk i from all-ones state
            (with the masked-tail / f-handoff emission machinery)

and the host composes Z per sequence with rank-1 cross-approximation dots
  lam_i = E^T omega_i
  x = alpha_1;  for i = 2..j-1:  x = alpha_i * (lam_i.x)/sum(lam_i)
  Z = lam_j . x      (j = chunk containing len; log-scaled, f64)

Each chunk's fwd+bwd passes stack into one 128-partition chain (state
[u; c], block-diag weights [[E,0],[0,E^T]] — same stationary matrix for
everything). K=16 chunks -> 16 chains of 64 rounds. Chains are packed G=4
per instruction group: one matmul [128 x 128 x 256] + one elementwise
multiply per group per wave, 4 groups running concurrently across engines.
The wall-clock is then engine-throughput-bound (~100us), not chain-latency
bound (v1: 1024 serial rounds x ~650ns = 662us; v2: 513 x ~600ns).

All boundary conditions ride on host-fabricated emissions (kappa trick,
see kernel_v2); bf16 state/weights/emissions, fp32 PSUM.
"""

import os
import sys

import numpy as np

S = 1024           # sequence length
N = 64             # n_labels
L = 66             # n_labels + 2 (START, END)
B = 512            # batch
NCORES = 8
BL = B // NCORES   # 64 sequences per core
NS = 2 * N         # stacked state rows [fwd; bwd]
K = int(os.environ.get("V3_K", "16"))   # time chunks (= independent chains)
CL = S // K        # rounds per chain
G = int(os.environ.get("V3_G", "4"))    # chains per instruction group
NG = K // G        # instruction groups
GW = G * BL        # moving columns per grouped instruction
C0 = 4.66          # emission centering constant (~log(64*e^0.5))
NEG = -1000.0
MINLEN = S // 2    # guaranteed minimum sequence length
DMAW = 8           # waves per emission DMA chunk
# mul engine per group: 'v' = DVE (vector), 'g' = GpSimd (pool)
ENGS = os.environ.get("V3_ENGS", "v" * NG)

_BASS_PATHS = (
    "/opt/trn_rl_repo",
    os.path.expanduser("~/.axon_site/_ro/trn_rl_repo"),
)


def _import_bass():
    try:
        import concourse.bass  # noqa: F401
    except ImportError:
        for p in _BASS_PATHS:
            if os.path.isdir(p) and p not in sys.path:
                sys.path.insert(0, p)
    import concourse.bass as bass
    import concourse.bacc as bacc
    import concourse.mybir as mybir
    import concourse.tile as tile
    from concourse import bass_utils
    return bass, mybir, tile, bass_utils, bacc


def _bf16():
    _, mybir, _, _, _ = _import_bass()
    return mybir.dt.np(mybir.dt.bfloat16)


_PROGRAM_CACHE = {}


def build_program():
    """Build the (input-independent) Bass program; returns nc."""
    key = ("nc", ENGS)
    if key in _PROGRAM_CACHE:
        return _PROGRAM_CACHE[key]
    bass, mybir, tile, _, bacc = _import_bass()
    from contextlib import ExitStack

    f32 = mybir.dt.float32
    bf16 = mybir.dt.bfloat16

    nc = bacc.Bacc("TRN2", target_bir_lowering=False, debug=False,
                   enable_asserts=False)
    el = nc.dram_tensor("el", [NS, CL, K * BL], bf16, kind="ExternalInput").ap()
    wmat = nc.dram_tensor("wmat", [NS, NS], bf16, kind="ExternalInput").ap()
    outs = nc.dram_tensor("outs", [NS, K * BL], f32, kind="ExternalOutput").ap()

    with tile.TileContext(nc) as tc, ExitStack() as ctx:
        consts = ctx.enter_context(tc.tile_pool(name="consts", bufs=1))
        els = ctx.enter_context(tc.tile_pool(name="els", bufs=1))
        vs = ctx.enter_context(tc.tile_pool(name="vs", bufs=2))
        opool = ctx.enter_context(tc.tile_pool(name="opool", bufs=1))
        qpool = ctx.enter_context(tc.tile_pool(name="qpool", bufs=2, space="PSUM"))

        wsb = consts.tile([NS, NS], bf16)
        nc.sync.dma_start(out=wsb, in_=wmat)
        el_t = []
        for c in range(CL // DMAW):
            t = els.tile([NS, DMAW, K * BL], bf16, tag=f"el{c}")
            nc.sync.dma_start(out=t, in_=el[:, c * DMAW:(c + 1) * DMAW, :])
            el_t.append(t)

        mul_eng = [nc.vector if e == "v" else nc.gpsimd for e in ENGS]
        s = []
        for g in range(NG):
            t = vs.tile([NS, GW], bf16, tag=f"s{g}")
            nc.vector.memset(t, 1.0)
            s.append(t)
        for tau in range(CL):
            for g in range(NG):
                q = qpool.tile([NS, GW], f32, tag=f"q{g}")
                nc.tensor.matmul(q, wsb, s[g], start=True, stop=True)
                s_new = vs.tile([NS, GW], bf16, tag=f"s{g}")
                mul_eng[g].tensor_mul(
                    s_new, el_t[tau // DMAW][:, tau % DMAW,
                                             g * GW:(g + 1) * GW], q)
                s[g] = s_new
        osb = opool.tile([NS, K * BL], f32, tag="out")
        for g in range(NG):
            nc.vector.tensor_copy(osb[:, g * GW:(g + 1) * GW], s[g])
        nc.sync.dma_start(out=outs, in_=osb)

    nc.compile()
    _PROGRAM_CACHE[key] = nc
    return nc


def _transition_mats(transition):
    """E, f, st, g_f, g_b in float64 from the [66,66] transition matrix."""
    T = np.asarray(transition, np.float64)
    E = np.exp(T[0:N, 0:N])          # E[to, frm]
    f = np.exp(T[L - 1, 0:N])        # into END
    st = np.exp(T[0:N, L - 2])       # from START
    g_f = E.sum(axis=1)              # E @ 1   (forward kappa correction)
    g_b = E.sum(axis=0)              # E^T @ 1 (backward kappa correction)
    return E, f, st, g_f, g_b


def make_wmat(transition):
    """Stationary block-diag matrix in lhsT layout [NS, NS]:
    out rows 0:64 = E u  ->  lhsT[k, i] = E[i, k];
    out rows 64:128 = E^T c -> lhsT[64+k, 64+i] = E[k, i]."""
    E, _, _, _, _ = _transition_mats(transition)
    wmat = np.zeros((NS, NS), np.float64)
    wmat[0:N, 0:N] = E.T
    wmat[N:NS, N:NS] = E
    return wmat.astype(_bf16())


def _fabricate_el(logits, transition, lengths):
    """Emission stream [NS, CL, K, B] float32 (pre-bf16, pre-core-slice)."""
    E, f, st, g_f, g_b = _transition_mats(transition)
    base = np.exp(np.asarray(logits, np.float32) - np.float32(C0))  # [B,S,N]
    baseT = base.transpose(1, 2, 0)                                 # [S,N,B]

    elf = baseT.copy()
    elf[0] = elf[0] * (st / g_f)[:, None].astype(np.float32)

    t_idx = np.arange(1, S + 1)
    hand = (t_idx[:, None] == lengths[None, :])                     # [S,B]
    mskd = (t_idx[:, None] > lengths[None, :])
    elb = baseT * np.where(hand[:, None, :], (f / g_b)[:, None],
                           1.0).astype(np.float32)
    elb = np.where(mskd[:, None, :],
                   (1.0 / g_b)[:, None].astype(np.float32), elb)

    elf_r = elf.reshape(K, CL, N, B)            # [k, tau] = t: k*CL+tau+1
    elb_r = elb.reshape(K, CL, N, B)[:, ::-1]   # [k, tau] = t: (k+1)*CL-tau
    el_all = np.concatenate([elf_r, elb_r], axis=2)  # [K, CL, NS, B]
    return el_all.transpose(2, 1, 0, 3)              # [NS, CL, K, B]


def _host_prep(logits, transition, predict_mask):
    """Returns (in_maps, lengths). Raises ValueError if inputs unsupported."""
    lengths = np.asarray(predict_mask, np.int64).sum(1)
    prefix = (np.asarray(predict_mask, np.int64)
              == (np.arange(S)[None, :] < lengths[:, None])).all()
    if not prefix or lengths.min() < MINLEN:
        raise ValueError("mask is not a contiguous prefix with len >= S/2")

    el_all = _fabricate_el(logits, transition, lengths)  # [NS, CL, K, B]
    wmat = make_wmat(transition)
    bf = _bf16()

    in_maps = []
    for c in range(NCORES):
        el_core = np.ascontiguousarray(
            el_all[:, :, :, c * BL:(c + 1) * BL]).reshape(NS, CL, K * BL)
        in_maps.append({"el": el_core.astype(bf), "wmat": wmat})
    return in_maps, lengths


def _host_gold(logits, transition, labels, predict_mask):
    T = np.asarray(transition, np.float64)
    lab = np.asarray(labels, np.int64)
    maskf = np.asarray(predict_mask, np.float64)
    logits64 = np.asarray(logits, np.float64)
    start, end = L - 2, L - 1
    unary = np.take_along_axis(logits64, lab[:, :, None], axis=2)[..., 0] * maskf
    labels_ext = np.concatenate(
        [np.full((B, 1), start), lab, np.full((B, 1), end)], 1)
    mask_ext = np.concatenate([np.ones((B, 1)), maskf, np.ones((B, 1))], 1)
    labels_m = np.where(mask_ext > 0, labels_ext, end).astype(np.int64)
    trn_scr = T[labels_m[:, 1:], labels_m[:, :-1]]
    mask2 = np.concatenate([np.ones((B, 1)), maskf], 1)
    return unary.sum(1) + (trn_scr * mask2).sum(1)


def _fallback_numpy(logits, transition, labels, predict_mask):
    """Pure-host reference implementation (only for unsupported inputs)."""
    logits = np.asarray(logits, np.float64)
    T = np.asarray(transition, np.float64)
    mask = np.asarray(predict_mask)
    Bn, Sn, n = logits.shape
    Ln_ = T.shape[0]
    start, end = Ln_ - 2, Ln_ - 1
    pads = np.full((Bn, Sn, 2), NEG)
    logits_p = np.concatenate([logits, pads], 2)
    alpha = np.full((Bn, Ln_), -100.0)
    alpha[:, start] = 0.0
    for t in range(Sn):
        mat = logits_p[:, t, :, None] + alpha[:, None, :] + T[None]
        m = mat.max(2, keepdims=True)
        a_n = (m[..., 0] + np.log(np.exp(mat - m).sum(2)))
        alpha = np.where(mask[:, t:t + 1] > 0, a_n, alpha)
    mm = (alpha + T[end][None]).max(1, keepdims=True)
    norm = mm[:, 0] + np.log(np.exp(alpha + T[end][None] - mm).sum(1))
    gold = _host_gold(logits, T, labels, mask)
    return (norm - gold).astype(np.float32)


def run_device(in_maps, trace=False, **kw):
    _, _, _, bass_utils, _ = _import_bass()
    nc = build_program()
    return bass_utils.run_bass_kernel_spmd(
        nc, in_maps, core_ids=list(range(NCORES)), trace=trace, **kw)


def _norm_from_outs(outs_list, transition, lengths):
    """ln Z + C0*len per sequence from the per-core [NS, K*BL] outputs."""
    E, _, _, _, _ = _transition_mats(transition)
    o = np.stack([np.asarray(o, np.float64).reshape(NS, K, BL)
                  for o in outs_list], -1)                  # [NS, K, BL, NC]
    alphas = o[0:N].transpose(1, 0, 3, 2).reshape(K, N, B)
    omegas = o[N:NS].transpose(1, 0, 3, 2).reshape(K, N, B)
    lams = np.einsum("ij,kjb->kib", E.T, omegas)            # lam_i = E^T omega_i
    j_of = (np.asarray(lengths, np.int64) - 1) // CL        # 0-based chunk
    lnZ = np.zeros(B)
    ln_scale = np.zeros(B)
    x = alphas[0].copy()
    for i in range(1, K):
        lam = lams[i]
        dot = (lam * x).sum(0)
        fin = j_of == i
        cont = j_of > i
        lnZ[fin] = np.log(dot[fin]) + ln_scale[fin]
        ln_scale[cont] += np.log(dot[cont]) - np.log(lam.sum(0)[cont])
        x = alphas[i]
    return lnZ + C0 * lengths


def kernel(logits, transition, labels, predict_mask):
    logits = np.asarray(logits)
    transition = np.asarray(transition)
    labels = np.asarray(labels)
    predict_mask = np.asarray(predict_mask)
    assert logits.shape == (B, S, N) and transition.shape == (L, L)

    try:
        in_maps, lengths = _host_prep(logits, transition, predict_mask)
    except ValueError:
        return _fallback_numpy(logits, transition, labels, predict_mask)

    res = run_device(in_maps)
    norm = _norm_from_outs(
        [res.results[c]["outs"] for c in range(NCORES)], transition, lengths)
    gold = _host_gold(logits, transition, labels, predict_mask)
    return (norm - gold).astype(np.float32)


# revision 7
# speedup vs baseline: 1.8127x; 1.8127x over previous
"""Trainium2 Bass kernel for linear-chain CRF NLL — chunked rank-1 scan.

v2 halved the serial depth with a forward/backward meet-in-the-middle; this
version removes the latency wall entirely. The chunk transfer matrices
P_i = prod_{t in chunk} (D_t E) are numerically rank-1 (E = exp(T) with
T ~ U[-0.1,0.1] is a small perturbation of the all-ones matrix, so a
64-step product contracts off-rank components by ~1e-50; validated to f32
roundoff in _sim_v3.py). Hence the time axis splits into K independent
chunks; the device only computes, per chunk and per sequence:

  alpha_i = forward pass over chunk i from all-ones state
  omega_i = backward pass over chunk i from all-ones state
            (with the masked-tail / f-handoff emission machinery)

and the host composes Z per sequence with rank-1 cross-approximation dots
  lam_i = E^T omega_i
  x = alpha_1;  for i = 2..j-1:  x = alpha_i * (lam_i.x)/sum(lam_i)
  Z = lam_j . x      (j = chunk containing len; log-scaled, f64)

Each chunk's fwd+bwd passes stack into one 128-partition chain (state
[u; c], block-diag weights [[E,0],[0,E^T]] — same stationary matrix for
everything). K=16 chunks -> 16 chains of 64 rounds. Chains are packed G=4
per instruction group: one matmul [128 x 128 x 256] + one elementwise
multiply per group per wave, 4 groups running concurrently across engines.
The wall-clock is then engine-throughput-bound (~100us), not chain-latency
bound (v1: 1024 serial rounds x ~650ns = 662us; v2: 513 x ~600ns).

All boundary conditions ride on host-fabricated emissions (kappa trick,
see kernel_v2); bf16 state/weights/emissions, fp32 PSUM.
"""

import os
import sys

import numpy as np

S = 1024           # sequence length
N = 64             # n_labels
L = 66             # n_labels + 2 (START, END)
B = 512            # batch
NCORES = 8
BL = B // NCORES   # 64 sequences per core
NS = 2 * N         # stacked state rows [fwd; bwd]
K = int(os.environ.get("V3_K", "16"))   # time chunks (= independent chains)
CL = S // K        # rounds per chain
G = int(os.environ.get("V3_G", "4"))    # chains per instruction group
NG = K // G        # instruction groups
GW = G * BL        # moving columns per grouped instruction
C0 = 4.66          # emission centering constant (~log(64*e^0.5))
NEG = -1000.0
MINLEN = S // 2    # guaranteed minimum sequence length
DMAW = 8           # waves per emission DMA chunk
# mul engine per group: 'v' = DVE (vector), 'g' = GpSimd (pool)
ENGS = os.environ.get("V3_ENGS", "v" * NG)

_BASS_PATHS = (
    "/opt/trn_rl_repo",
    os.path.expanduser("~/.axon_site/_ro/trn_rl_repo"),
)


def _import_bass():
    try:
        import concourse.bass  # noqa: F401
    except ImportError:
        for p in _BASS_PATHS:
            if os.path.isdir(p) and p not in sys.path:
                sys.path.insert(0, p)
    import concourse.bass as bass
    import concourse.bacc as bacc
    import concourse.mybir as mybir
    import concourse.tile as tile
    from concourse import bass_utils
    return bass, mybir, tile, bass_utils, bacc


def _bf16():
    _, mybir, _, _, _ = _import_bass()
    return mybir.dt.np(mybir.dt.bfloat16)


_PROGRAM_CACHE = {}


def build_program():
    """Build the (input-independent) Bass program; returns nc."""
    key = ("nc", ENGS)
    if key in _PROGRAM_CACHE:
        return _PROGRAM_CACHE[key]
    bass, mybir, tile, _, bacc = _import_bass()
    from contextlib import ExitStack

    f32 = mybir.dt.float32
    bf16 = mybir.dt.bfloat16

    nc = bacc.Bacc("TRN2", target_bir_lowering=False, debug=False,
                   enable_asserts=False)
    el = nc.dram_tensor("el", [NS, CL, K * BL], bf16, kind="ExternalInput").ap()
    wmat = nc.dram_tensor("wmat", [NS, NS], bf16, kind="ExternalInput").ap()
    outs = nc.dram_tensor("outs", [NS, K * BL], f32, kind="ExternalOutput").ap()

    with tile.TileContext(nc) as tc, ExitStack() as ctx:
        consts = ctx.enter_context(tc.tile_pool(name="consts", bufs=1))
        els = ctx.enter_context(tc.tile_pool(name="els", bufs=1))
        vs = ctx.enter_context(tc.tile_pool(name="vs", bufs=3))
        opool = ctx.enter_context(tc.tile_pool(name="opool", bufs=1))
        qpool = ctx.enter_context(tc.tile_pool(name="qpool", bufs=2, space="PSUM"))

        wsb = consts.tile([NS, NS], bf16)
        nc.sync.dma_start(out=wsb, in_=wmat)
        # small leading chunks so wave 0 can start before the bulk arrives
        sizes = [1, 3, 4] + [DMAW] * ((CL - 8) // DMAW)
        assert sum(sizes) == CL
        el_t = []       # (tile, first_wave, n_waves) spans
        lo = 0
        for c, n in enumerate(sizes):
            t = els.tile([NS, n, K * BL], bf16, tag=f"el{c}")
            nc.sync.dma_start(out=t, in_=el[:, lo:lo + n, :])
            el_t.append((t, lo, n))
            lo += n

        mul_eng = [nc.vector if e == "v" else nc.gpsimd for e in ENGS]
        s = []
        for g in range(NG):
            t = vs.tile([NS, GW], bf16, tag=f"s{g}")
            nc.vector.memset(t, 1.0)
            s.append(t)
        chunk_of = []
        for t, lo, n in el_t:
            chunk_of += [(t, lo)] * n
        for tau in range(CL):
            et, lo = chunk_of[tau]
            for g in range(NG):
                q = qpool.tile([NS, GW], f32, tag=f"q{g}")
                nc.tensor.matmul(q, wsb, s[g], start=True, stop=True)
                s_new = vs.tile([NS, GW], bf16, tag=f"s{g}")
                mul_eng[g].tensor_mul(
                    s_new, et[:, tau - lo, g * GW:(g + 1) * GW], q)
                s[g] = s_new
        osb = opool.tile([NS, K * BL], f32, tag="out")
        for g in range(NG):
            nc.vector.tensor_copy(osb[:, g * GW:(g + 1) * GW], s[g])
        nc.sync.dma_start(out=outs, in_=osb)

    nc.compile()
    _PROGRAM_CACHE[key] = nc
    return nc


def _transition_mats(transition):
    """E, f, st, g_f, g_b in float64 from the [66,66] transition matrix."""
    T = np.asarray(transition, np.float64)
    E = np.exp(T[0:N, 0:N])          # E[to, frm]
    f = np.exp(T[L - 1, 0:N])        # into END
    st = np.exp(T[0:N, L - 2])       # from START
    g_f = E.sum(axis=1)              # E @ 1   (forward kappa correction)
    g_b = E.sum(axis=0)              # E^T @ 1 (backward kappa correction)
    return E, f, st, g_f, g_b


def make_wmat(transition):
    """Stationary block-diag matrix in lhsT layout [NS, NS]:
    out rows 0:64 = E u  ->  lhsT[k, i] = E[i, k];
    out rows 64:128 = E^T c -> lhsT[64+k, 64+i] = E[k, i]."""
    E, _, _, _, _ = _transition_mats(transition)
    wmat = np.zeros((NS, NS), np.float64)
    wmat[0:N, 0:N] = E.T
    wmat[N:NS, N:NS] = E
    return wmat.astype(_bf16())


def _fabricate_el(logits, transition, lengths):
    """Emission stream [NS, CL, K, B] float32 (pre-bf16, pre-core-slice)."""
    E, f, st, g_f, g_b = _transition_mats(transition)
    base = np.exp(np.asarray(logits, np.float32) - np.float32(C0))  # [B,S,N]
    baseT = base.transpose(1, 2, 0)                                 # [S,N,B]

    elf = baseT.copy()
    elf[0] = elf[0] * (st / g_f)[:, None].astype(np.float32)

    t_idx = np.arange(1, S + 1)
    hand = (t_idx[:, None] == lengths[None, :])                     # [S,B]
    mskd = (t_idx[:, None] > lengths[None, :])
    elb = baseT * np.where(hand[:, None, :], (f / g_b)[:, None],
                           1.0).astype(np.float32)
    elb = np.where(mskd[:, None, :],
                   (1.0 / g_b)[:, None].astype(np.float32), elb)

    elf_r = elf.reshape(K, CL, N, B)            # [k, tau] = t: k*CL+tau+1
    elb_r = elb.reshape(K, CL, N, B)[:, ::-1]   # [k, tau] = t: (k+1)*CL-tau
    el_all = np.concatenate([elf_r, elb_r], axis=2)  # [K, CL, NS, B]
    return el_all.transpose(2, 1, 0, 3)              # [NS, CL, K, B]


def _host_prep(logits, transition, predict_mask):
    """Returns (in_maps, lengths). Raises ValueError if inputs unsupported."""
    lengths = np.asarray(predict_mask, np.int64).sum(1)
    prefix = (np.asarray(predict_mask, np.int64)
              == (np.arange(S)[None, :] < lengths[:, None])).all()
    if not prefix or lengths.min() < MINLEN:
        raise ValueError("mask is not a contiguous prefix with len >= S/2")

    el_all = _fabricate_el(logits, transition, lengths)  # [NS, CL, K, B]
    wmat = make_wmat(transition)
    bf = _bf16()

    in_maps = []
    for c in range(NCORES):
        el_core = np.ascontiguousarray(
            el_all[:, :, :, c * BL:(c + 1) * BL]).reshape(NS, CL, K * BL)
        in_maps.append({"el": el_core.astype(bf), "wmat": wmat})
    return in_maps, lengths


def _host_gold(logits, transition, labels, predict_mask):
    T = np.asarray(transition, np.float64)
    lab = np.asarray(labels, np.int64)
    maskf = np.asarray(predict_mask, np.float64)
    logits64 = np.asarray(logits, np.float64)
    start, end = L - 2, L - 1
    unary = np.take_along_axis(logits64, lab[:, :, None], axis=2)[..., 0] * maskf
    labels_ext = np.concatenate(
        [np.full((B, 1), start), lab, np.full((B, 1), end)], 1)
    mask_ext = np.concatenate([np.ones((B, 1)), maskf, np.ones((B, 1))], 1)
    labels_m = np.where(mask_ext > 0, labels_ext, end).astype(np.int64)
    trn_scr = T[labels_m[:, 1:], labels_m[:, :-1]]
    mask2 = np.concatenate([np.ones((B, 1)), maskf], 1)
    return unary.sum(1) + (trn_scr * mask2).sum(1)


def _fallback_numpy(logits, transition, labels, predict_mask):
    """Pure-host reference implementation (only for unsupported inputs)."""
    logits = np.asarray(logits, np.float64)
    T = np.asarray(transition, np.float64)
    mask = np.asarray(predict_mask)
    Bn, Sn, n = logits.shape
    Ln_ = T.shape[0]
    start, end = Ln_ - 2, Ln_ - 1
    pads = np.full((Bn, Sn, 2), NEG)
    logits_p = np.concatenate([logits, pads], 2)
    alpha = np.full((Bn, Ln_), -100.0)
    alpha[:, start] = 0.0
    for t in range(Sn):
        mat = logits_p[:, t, :, None] + alpha[:, None, :] + T[None]
        m = mat.max(2, keepdims=True)
        a_n = (m[..., 0] + np.log(np.exp(mat - m).sum(2)))
        alpha = np.where(mask[:, t:t + 1] > 0, a_n, alpha)
    mm = (alpha + T[end][None]).max(1, keepdims=True)
    norm = mm[:, 0] + np.log(np.exp(alpha + T[end][None] - mm).sum(1))
    gold = _host_gold(logits, T, labels, mask)
    return (norm - gold).astype(np.float32)


def run_device(in_maps, trace=False, **kw):
    _, _, _, bass_utils, _ = _import_bass()
    nc = build_program()
    return bass_utils.run_bass_kernel_spmd(
        nc, in_maps, core_ids=list(range(NCORES)), trace=trace, **kw)


def _norm_from_outs(outs_list, transition, lengths):
    """ln Z + C0*len per sequence from the per-core [NS, K*BL] outputs."""
    E, _, _, _, _ = _transition_mats(transition)
    o = np.stack([np.asarray(o, np.float64).reshape(NS, K, BL)
                  for o in outs_list], -1)                  # [NS, K, BL, NC]
    alphas = o[0:N].transpose(1, 0, 3, 2).reshape(K, N, B)
    omegas = o[N:NS].transpose(1, 0, 3, 2).reshape(K, N, B)
    lams = np.einsum("ij,kjb->kib", E.T, omegas)            # lam_i = E^T omega_i
    j_of = (np.asarray(lengths, np.int64) - 1) // CL        # 0-based chunk
    lnZ = np.zeros(B)
    ln_scale = np.zeros(B)
    x = alphas[0].copy()
    for i in range(1, K):
        lam = lams[i]
        dot = (lam * x).sum(0)
        fin = j_of == i
        cont = j_of > i
        lnZ[fin] = np.log(dot[fin]) + ln_scale[fin]
        ln_scale[cont] += np.log(dot[cont]) - np.log(lam.sum(0)[cont])
        x = alphas[i]
    return lnZ + C0 * lengths


def kernel(logits, transition, labels, predict_mask):
    logits = np.asarray(logits)
    transition = np.asarray(transition)
    labels = np.asarray(labels)
    predict_mask = np.asarray(predict_mask)
    assert logits.shape == (B, S, N) and transition.shape == (L, L)

    try:
        in_maps, lengths = _host_prep(logits, transition, predict_mask)
    except ValueError:
        return _fallback_numpy(logits, transition, labels, predict_mask)

    res = run_device(in_maps)
    norm = _norm_from_outs(
        [res.results[c]["outs"] for c in range(NCORES)], transition, lengths)
    gold = _host_gold(logits, transition, labels, predict_mask)
    return (norm - gold).astype(np.float32)


# revision 9
# speedup vs baseline: 1.8306x; 1.0099x over previous
"""Trainium2 Bass kernel for linear-chain CRF NLL — chunked rank-1 scan.

v2 halved the serial depth with a forward/backward meet-in-the-middle; this
version removes the latency wall entirely. The chunk transfer matrices
P_i = prod_{t in chunk} (D_t E) are numerically rank-1 (E = exp(T) with
T ~ U[-0.1,0.1] is a small perturbation of the all-ones matrix, so a
64-step product contracts off-rank components by ~1e-50; validated to f32
roundoff in _sim_v3.py). Hence the time axis splits into K independent
chunks; the device only computes, per chunk and per sequence:

  alpha_i = forward pass over chunk i from all-ones state
  omega_i = backward pass over chunk i from all-ones state
            (with the masked-tail / f-handoff emission machinery)

and the host composes Z per sequence with rank-1 cross-approximation dots
  lam_i = E^T omega_i
  x = alpha_1;  for i = 2..j-1:  x = alpha_i * (lam_i.x)/sum(lam_i)
  Z = lam_j . x      (j = chunk containing len; log-scaled, f64)

Each chunk's fwd+bwd passes stack into one 128-partition chain (state
[u; c], block-diag weights [[E,0],[0,E^T]] — same stationary matrix for
everything). K=32 chunks -> 32 chains of 32 rounds, packed G=8 chains per
instruction group: one matmul [128 x 128 x 512] + one elementwise multiply
per group per wave, 4 groups rotating so PE and DVE stay saturated (big
instructions amortize the ~125ns fixed LDWEIGHTS / PSUM-access costs).
The wall-clock is engine-throughput-bound (~99us measured), not chain-
latency bound (v1: 1024 serial rounds x ~650ns = 662us; v2: 513 x ~560ns
= 289us). GpSimd cannot read PSUM on TRN2 and the Scalar-copy bounce is
slower than the DVE's direct PSUM multiply, so all multiplies go to DVE.

All boundary conditions ride on host-fabricated emissions (kappa trick,
see kernel_v2); bf16 state/weights/emissions, fp32 PSUM.
"""

import os
import sys

import numpy as np

S = 1024           # sequence length
N = 64             # n_labels
L = 66             # n_labels + 2 (START, END)
B = 512            # batch
NCORES = 8
BL = B // NCORES   # 64 sequences per core
NS = 2 * N         # stacked state rows [fwd; bwd]
K = int(os.environ.get("V3_K", "32"))   # time chunks (= independent chains)
CL = S // K        # rounds per chain
G = int(os.environ.get("V3_G", "8"))    # chains per instruction group
NG = K // G        # instruction groups
GW = G * BL        # moving columns per grouped instruction
C0 = 4.66          # emission centering constant (~log(64*e^0.5))
NEG = -1000.0
MINLEN = S // 2    # guaranteed minimum sequence length
DMAW = 8           # waves per emission DMA chunk
# mul engine per group: 'v' = DVE (vector), 'g' = GpSimd (pool)
ENGS = os.environ.get("V3_ENGS", "v" * NG)

_BASS_PATHS = (
    "/opt/trn_rl_repo",
    os.path.expanduser("~/.axon_site/_ro/trn_rl_repo"),
)


def _import_bass():
    try:
        import concourse.bass  # noqa: F401
    except ImportError:
        for p in _BASS_PATHS:
            if os.path.isdir(p) and p not in sys.path:
                sys.path.insert(0, p)
    import concourse.bass as bass
    import concourse.bacc as bacc
    import concourse.mybir as mybir
    import concourse.tile as tile
    from concourse import bass_utils
    return bass, mybir, tile, bass_utils, bacc


def _bf16():
    _, mybir, _, _, _ = _import_bass()
    return mybir.dt.np(mybir.dt.bfloat16)


_PROGRAM_CACHE = {}


def build_program():
    """Build the (input-independent) Bass program; returns nc."""
    key = ("nc", ENGS)
    if key in _PROGRAM_CACHE:
        return _PROGRAM_CACHE[key]
    bass, mybir, tile, _, bacc = _import_bass()
    from contextlib import ExitStack

    f32 = mybir.dt.float32
    bf16 = mybir.dt.bfloat16

    nc = bacc.Bacc("TRN2", target_bir_lowering=False, debug=False,
                   enable_asserts=False)
    el = nc.dram_tensor("el", [NS, CL, K * BL], bf16, kind="ExternalInput").ap()
    wmat = nc.dram_tensor("wmat", [NS, NS], bf16, kind="ExternalInput").ap()
    outs = nc.dram_tensor("outs", [NS, K * BL], f32, kind="ExternalOutput").ap()

    with tile.TileContext(nc) as tc, ExitStack() as ctx:
        consts = ctx.enter_context(tc.tile_pool(name="consts", bufs=1))
        els = ctx.enter_context(tc.tile_pool(name="els", bufs=1))
        vs = ctx.enter_context(tc.tile_pool(name="vs", bufs=3))
        opool = ctx.enter_context(tc.tile_pool(name="opool", bufs=1))
        qpool = ctx.enter_context(tc.tile_pool(name="qpool", bufs=2, space="PSUM"))

        wsb = consts.tile([NS, NS], bf16)
        nc.sync.dma_start(out=wsb, in_=wmat)
        # small leading chunks so wave 0 can start before the bulk arrives
        sizes = [1, 3, 4] + [DMAW] * ((CL - 8) // DMAW)
        assert sum(sizes) == CL
        el_t = []       # (tile, first_wave, n_waves) spans
        lo = 0
        for c, n in enumerate(sizes):
            t = els.tile([NS, n, K * BL], bf16, tag=f"el{c}")
            nc.sync.dma_start(out=t, in_=el[:, lo:lo + n, :])
            el_t.append((t, lo, n))
            lo += n

        mul_eng = [nc.vector if e == "v" else nc.gpsimd for e in ENGS]
        s = []
        for g in range(NG):
            t = vs.tile([NS, GW], bf16, tag=f"s{g}")
            nc.vector.memset(t, 1.0)
            s.append(t)
        chunk_of = []
        for t, lo, n in el_t:
            chunk_of += [(t, lo)] * n
        for tau in range(CL):
            et, lo = chunk_of[tau]
            for g in range(NG):
                q = qpool.tile([NS, GW], f32, tag=f"q{g}")
                nc.tensor.matmul(q, wsb, s[g], start=True, stop=True)
                s_new = vs.tile([NS, GW], bf16, tag=f"s{g}")
                mul_eng[g].tensor_mul(
                    s_new, et[:, tau - lo, g * GW:(g + 1) * GW], q)
                s[g] = s_new
        osb = opool.tile([NS, K * BL], f32, tag="out")
        for g in range(NG):
            nc.vector.tensor_copy(osb[:, g * GW:(g + 1) * GW], s[g])
        nc.sync.dma_start(out=outs, in_=osb)

    nc.compile()
    _PROGRAM_CACHE[key] = nc
    return nc


def _transition_mats(transition):
    """E, f, st, g_f, g_b in float64 from the [66,66] transition matrix."""
    T = np.asarray(transition, np.float64)
    E = np.exp(T[0:N, 0:N])          # E[to, frm]
    f = np.exp(T[L - 1, 0:N])        # into END
    st = np.exp(T[0:N, L - 2])       # from START
    g_f = E.sum(axis=1)              # E @ 1   (forward kappa correction)
    g_b = E.sum(axis=0)              # E^T @ 1 (backward kappa correction)
    return E, f, st, g_f, g_b


def make_wmat(transition):
    """Stationary block-diag matrix in lhsT layout [NS, NS]:
    out rows 0:64 = E u  ->  lhsT[k, i] = E[i, k];
    out rows 64:128 = E^T c -> lhsT[64+k, 64+i] = E[k, i]."""
    E, _, _, _, _ = _transition_mats(transition)
    wmat = np.zeros((NS, NS), np.float64)
    wmat[0:N, 0:N] = E.T
    wmat[N:NS, N:NS] = E
    return wmat.astype(_bf16())


def _fabricate_el(logits, transition, lengths):
    """Emission stream [NS, CL, K, B] float32 (pre-bf16, pre-core-slice)."""
    E, f, st, g_f, g_b = _transition_mats(transition)
    base = np.exp(np.asarray(logits, np.float32) - np.float32(C0))  # [B,S,N]
    baseT = base.transpose(1, 2, 0)                                 # [S,N,B]

    elf = baseT.copy()
    elf[0] = elf[0] * (st / g_f)[:, None].astype(np.float32)

    t_idx = np.arange(1, S + 1)
    hand = (t_idx[:, None] == lengths[None, :])                     # [S,B]
    mskd = (t_idx[:, None] > lengths[None, :])
    elb = baseT * np.where(hand[:, None, :], (f / g_b)[:, None],
                           1.0).astype(np.float32)
    elb = np.where(mskd[:, None, :],
                   (1.0 / g_b)[:, None].astype(np.float32), elb)

    elf_r = elf.reshape(K, CL, N, B)            # [k, tau] = t: k*CL+tau+1
    elb_r = elb.reshape(K, CL, N, B)[:, ::-1]   # [k, tau] = t: (k+1)*CL-tau
    el_all = np.concatenate([elf_r, elb_r], axis=2)  # [K, CL, NS, B]
    return el_all.transpose(2, 1, 0, 3)              # [NS, CL, K, B]


def _host_prep(logits, transition, predict_mask):
    """Returns (in_maps, lengths). Raises ValueError if inputs unsupported."""
    lengths = np.asarray(predict_mask, np.int64).sum(1)
    prefix = (np.asarray(predict_mask, np.int64)
              == (np.arange(S)[None, :] < lengths[:, None])).all()
    if not prefix or lengths.min() < MINLEN:
        raise ValueError("mask is not a contiguous prefix with len >= S/2")

    el_all = _fabricate_el(logits, transition, lengths)  # [NS, CL, K, B]
    wmat = make_wmat(transition)
    bf = _bf16()

    in_maps = []
    for c in range(NCORES):
        el_core = np.ascontiguousarray(
            el_all[:, :, :, c * BL:(c + 1) * BL]).reshape(NS, CL, K * BL)
        in_maps.append({"el": el_core.astype(bf), "wmat": wmat})
    return in_maps, lengths


def _host_gold(logits, transition, labels, predict_mask):
    T = np.asarray(transition, np.float64)
    lab = np.asarray(labels, np.int64)
    maskf = np.asarray(predict_mask, np.float64)
    logits64 = np.asarray(logits, np.float64)
    start, end = L - 2, L - 1
    unary = np.take_along_axis(logits64, lab[:, :, None], axis=2)[..., 0] * maskf
    labels_ext = np.concatenate(
        [np.full((B, 1), start), lab, np.full((B, 1), end)], 1)
    mask_ext = np.concatenate([np.ones((B, 1)), maskf, np.ones((B, 1))], 1)
    labels_m = np.where(mask_ext > 0, labels_ext, end).astype(np.int64)
    trn_scr = T[labels_m[:, 1:], labels_m[:, :-1]]
    mask2 = np.concatenate([np.ones((B, 1)), maskf], 1)
    return unary.sum(1) + (trn_scr * mask2).sum(1)


def _fallback_numpy(logits, transition, labels, predict_mask):
    """Pure-host reference implementation (only for unsupported inputs)."""
    logits = np.asarray(logits, np.float64)
    T = np.asarray(transition, np.float64)
    mask = np.asarray(predict_mask)
    Bn, Sn, n = logits.shape
    Ln_ = T.shape[0]
    start, end = Ln_ - 2, Ln_ - 1
    pads = np.full((Bn, Sn, 2), NEG)
    logits_p = np.concatenate([logits, pads], 2)
    alpha = np.full((Bn, Ln_), -100.0)
    alpha[:, start] = 0.0
    for t in range(Sn):
        mat = logits_p[:, t, :, None] + alpha[:, None, :] + T[None]
        m = mat.max(2, keepdims=True)
        a_n = (m[..., 0] + np.log(np.exp(mat - m).sum(2)))
        alpha = np.where(mask[:, t:t + 1] > 0, a_n, alpha)
    mm = (alpha + T[end][None]).max(1, keepdims=True)
    norm = mm[:, 0] + np.log(np.exp(alpha + T[end][None] - mm).sum(1))
    gold = _host_gold(logits, T, labels, mask)
    return (norm - gold).astype(np.float32)


def run_device(in_maps, trace=False, **kw):
    _, _, _, bass_utils, _ = _import_bass()
    nc = build_program()
    return bass_utils.run_bass_kernel_spmd(
        nc, in_maps, core_ids=list(range(NCORES)), trace=trace, **kw)


def _norm_from_outs(outs_list, transition, lengths):
    """ln Z + C0*len per sequence from the per-core [NS, K*BL] outputs."""
    E, _, _, _, _ = _transition_mats(transition)
    o = np.stack([np.asarray(o, np.float64).reshape(NS, K, BL)
                  for o in outs_list], -1)                  # [NS, K, BL, NC]
    alphas = o[0:N].transpose(1, 0, 3, 2).reshape(K, N, B)
    omegas = o[N:NS].transpose(1, 0, 3, 2).reshape(K, N, B)
    lams = np.einsum("ij,kjb->kib", E.T, omegas)            # lam_i = E^T omega_i
    j_of = (np.asarray(lengths, np.int64) - 1) // CL        # 0-based chunk
    lnZ = np.zeros(B)
    ln_scale = np.zeros(B)
    x = alphas[0].copy()
    for i in range(1, K):
        lam = lams[i]
        dot = (lam * x).sum(0)
        fin = j_of == i
        cont = j_of > i
        lnZ[fin] = np.log(dot[fin]) + ln_scale[fin]
        ln_scale[cont] += np.log(dot[cont]) - np.log(lam.sum(0)[cont])
        x = alphas[i]
    return lnZ + C0 * lengths


def kernel(logits, transition, labels, predict_mask):
    logits = np.asarray(logits)
    transition = np.asarray(transition)
    labels = np.asarray(labels)
    predict_mask = np.asarray(predict_mask)
    assert logits.shape == (B, S, N) and transition.shape == (L, L)

    try:
        in_maps, lengths = _host_prep(logits, transition, predict_mask)
    except ValueError:
        return _fallback_numpy(logits, transition, labels, predict_mask)

    res = run_device(in_maps)
    norm = _norm_from_outs(
        [res.results[c]["outs"] for c in range(NCORES)], transition, lengths)
    gold = _host_gold(logits, transition, labels, predict_mask)
    return (norm - gold).astype(np.float32)


# revision 11
# speedup vs baseline: 1.8358x; 1.0029x over previous
"""Trainium2 Bass kernel for linear-chain CRF NLL — chunked rank-1 scan.

v2 halved the serial depth with a forward/backward meet-in-the-middle; this
version removes the latency wall entirely. The chunk transfer matrices
P_i = prod_{t in chunk} (D_t E) are numerically rank-1 (E = exp(T) with
T ~ U[-0.1,0.1] is a small perturbation of the all-ones matrix, so a
64-step product contracts off-rank components by ~1e-50; validated to f32
roundoff in _sim_v3.py). Hence the time axis splits into K independent
chunks; the device only computes, per chunk and per sequence:

  alpha_i = forward pass over chunk i from all-ones state
  omega_i = backward pass over chunk i from all-ones state
            (with the masked-tail / f-handoff emission machinery)

and the host composes Z per sequence with rank-1 cross-approximation dots
  lam_i = E^T omega_i
  x = alpha_1;  for i = 2..j-1:  x = alpha_i * (lam_i.x)/sum(lam_i)
  Z = lam_j . x      (j = chunk containing len; log-scaled, f64)

Each chunk's fwd+bwd passes stack into one 128-partition chain (state
[u; c], block-diag weights [[E,0],[0,E^T]] — same stationary matrix for
everything). K=32 chunks -> 32 chains of 32 rounds, packed G=8 chains per
instruction group: one matmul [128 x 128 x 512] + one elementwise multiply
per group per wave, 4 groups rotating so PE and DVE stay saturated (big
instructions amortize the ~125ns fixed LDWEIGHTS / PSUM-access costs).
The wall-clock is engine-throughput-bound (~99us measured), not chain-
latency bound (v1: 1024 serial rounds x ~650ns = 662us; v2: 513 x ~560ns
= 289us). GpSimd cannot read PSUM on TRN2 and the Scalar-copy bounce is
slower than the DVE's direct PSUM multiply, so all multiplies go to DVE.

All boundary conditions ride on host-fabricated emissions (kappa trick,
see kernel_v2); bf16 state/weights/emissions, fp32 PSUM.
"""

import os
import sys

import numpy as np

S = 1024           # sequence length
N = 64             # n_labels
L = 66             # n_labels + 2 (START, END)
B = 512            # batch
NCORES = 8
BL = B // NCORES   # 64 sequences per core
NS = 2 * N         # stacked state rows [fwd; bwd]
K = int(os.environ.get("V3_K", "32"))   # time chunks (= independent chains)
CL = S // K        # rounds per chain
G = int(os.environ.get("V3_G", "8"))    # chains per instruction group
NG = K // G        # instruction groups
GW = G * BL        # moving columns per grouped instruction
C0 = 4.66          # emission centering constant (~log(64*e^0.5))
NEG = -1000.0
MINLEN = S // 2    # guaranteed minimum sequence length
DMAW = 8           # waves per emission DMA chunk
# mul engine per group: 'v' = DVE (vector), 'g' = GpSimd (pool)
ENGS = os.environ.get("V3_ENGS", "v" * NG)

_BASS_PATHS = (
    "/opt/trn_rl_repo",
    os.path.expanduser("~/.axon_site/_ro/trn_rl_repo"),
)


def _import_bass():
    try:
        import concourse.bass  # noqa: F401
    except ImportError:
        for p in _BASS_PATHS:
            if os.path.isdir(p) and p not in sys.path:
                sys.path.insert(0, p)
    import concourse.bass as bass
    import concourse.bacc as bacc
    import concourse.mybir as mybir
    import concourse.tile as tile
    from concourse import bass_utils
    return bass, mybir, tile, bass_utils, bacc


def _bf16():
    _, mybir, _, _, _ = _import_bass()
    return mybir.dt.np(mybir.dt.bfloat16)


_PROGRAM_CACHE = {}


def build_program():
    """Build the (input-independent) Bass program; returns nc."""
    key = ("nc", ENGS)
    if key in _PROGRAM_CACHE:
        return _PROGRAM_CACHE[key]
    bass, mybir, tile, _, bacc = _import_bass()
    from contextlib import ExitStack

    f32 = mybir.dt.float32
    bf16 = mybir.dt.bfloat16

    nc = bacc.Bacc("TRN2", target_bir_lowering=False, debug=False,
                   enable_asserts=False)
    el = nc.dram_tensor("el", [NS, CL, K * BL], bf16, kind="ExternalInput").ap()
    wmat = nc.dram_tensor("wmat", [NS, NS], bf16, kind="ExternalInput").ap()
    outs = nc.dram_tensor("outs", [NS, K * BL], bf16, kind="ExternalOutput").ap()

    with tile.TileContext(nc) as tc, ExitStack() as ctx:
        consts = ctx.enter_context(tc.tile_pool(name="consts", bufs=1))
        els = ctx.enter_context(tc.tile_pool(name="els", bufs=1))
        vs = ctx.enter_context(tc.tile_pool(name="vs", bufs=3))
        opool = ctx.enter_context(tc.tile_pool(name="opool", bufs=1))
        qpool = ctx.enter_context(tc.tile_pool(name="qpool", bufs=2, space="PSUM"))

        wsb = consts.tile([NS, NS], bf16)
        nc.sync.dma_start(out=wsb, in_=wmat)
        # small leading chunks so wave 0 can start before the bulk arrives
        sizes = [1, 3, 4] + [DMAW] * ((CL - 8) // DMAW)
        assert sum(sizes) == CL
        el_t = []       # (tile, first_wave, n_waves) spans
        lo = 0
        for c, n in enumerate(sizes):
            t = els.tile([NS, n, K * BL], bf16, tag=f"el{c}")
            nc.sync.dma_start(out=t, in_=el[:, lo:lo + n, :])
            el_t.append((t, lo, n))
            lo += n

        mul_eng = [nc.vector if e == "v" else nc.gpsimd for e in ENGS]
        s = []
        for g in range(NG):
            t = vs.tile([NS, GW], bf16, tag=f"s{g}")
            nc.vector.memset(t, 1.0)
            s.append(t)
        chunk_of = []
        for t, lo, n in el_t:
            chunk_of += [(t, lo)] * n
        for tau in range(CL):
            et, lo = chunk_of[tau]
            for g in range(NG):
                q = qpool.tile([NS, GW], f32, tag=f"q{g}")
                nc.tensor.matmul(q, wsb, s[g], start=True, stop=True)
                s_new = vs.tile([NS, GW], bf16, tag=f"s{g}")
                mul_eng[g].tensor_mul(
                    s_new, et[:, tau - lo, g * GW:(g + 1) * GW], q)
                s[g] = s_new
        # DMA each group's final bf16 state out directly (no f32 conversion
        # copy); the host composes in f64 anyway. Each group's DMA can issue
        # as soon as that group's last multiply lands.
        for g in range(NG):
            nc.sync.dma_start(out=outs[:, g * GW:(g + 1) * GW], in_=s[g])

    nc.compile()
    _PROGRAM_CACHE[key] = nc
    return nc


def _transition_mats(transition):
    """E, f, st, g_f, g_b in float64 from the [66,66] transition matrix."""
    T = np.asarray(transition, np.float64)
    E = np.exp(T[0:N, 0:N])          # E[to, frm]
    f = np.exp(T[L - 1, 0:N])        # into END
    st = np.exp(T[0:N, L - 2])       # from START
    g_f = E.sum(axis=1)              # E @ 1   (forward kappa correction)
    g_b = E.sum(axis=0)              # E^T @ 1 (backward kappa correction)
    return E, f, st, g_f, g_b


def make_wmat(transition):
    """Stationary block-diag matrix in lhsT layout [NS, NS]:
    out rows 0:64 = E u  ->  lhsT[k, i] = E[i, k];
    out rows 64:128 = E^T c -> lhsT[64+k, 64+i] = E[k, i]."""
    E, _, _, _, _ = _transition_mats(transition)
    wmat = np.zeros((NS, NS), np.float64)
    wmat[0:N, 0:N] = E.T
    wmat[N:NS, N:NS] = E
    return wmat.astype(_bf16())


def _fabricate_el(logits, transition, lengths):
    """Emission stream [NS, CL, K, B] float32 (pre-bf16, pre-core-slice)."""
    E, f, st, g_f, g_b = _transition_mats(transition)
    base = np.exp(np.asarray(logits, np.float32) - np.float32(C0))  # [B,S,N]
    baseT = base.transpose(1, 2, 0)                                 # [S,N,B]

    elf = baseT.copy()
    elf[0] = elf[0] * (st / g_f)[:, None].astype(np.float32)

    t_idx = np.arange(1, S + 1)
    hand = (t_idx[:, None] == lengths[None, :])                     # [S,B]
    mskd = (t_idx[:, None] > lengths[None, :])
    elb = baseT * np.where(hand[:, None, :], (f / g_b)[:, None],
                           1.0).astype(np.float32)
    elb = np.where(mskd[:, None, :],
                   (1.0 / g_b)[:, None].astype(np.float32), elb)

    elf_r = elf.reshape(K, CL, N, B)            # [k, tau] = t: k*CL+tau+1
    elb_r = elb.reshape(K, CL, N, B)[:, ::-1]   # [k, tau] = t: (k+1)*CL-tau
    el_all = np.concatenate([elf_r, elb_r], axis=2)  # [K, CL, NS, B]
    return el_all.transpose(2, 1, 0, 3)              # [NS, CL, K, B]


def _host_prep(logits, transition, predict_mask):
    """Returns (in_maps, lengths). Raises ValueError if inputs unsupported."""
    lengths = np.asarray(predict_mask, np.int64).sum(1)
    prefix = (np.asarray(predict_mask, np.int64)
              == (np.arange(S)[None, :] < lengths[:, None])).all()
    if not prefix or lengths.min() < MINLEN:
        raise ValueError("mask is not a contiguous prefix with len >= S/2")

    el_all = _fabricate_el(logits, transition, lengths)  # [NS, CL, K, B]
    wmat = make_wmat(transition)
    bf = _bf16()

    in_maps = []
    for c in range(NCORES):
        el_core = np.ascontiguousarray(
            el_all[:, :, :, c * BL:(c + 1) * BL]).reshape(NS, CL, K * BL)
        in_maps.append({"el": el_core.astype(bf), "wmat": wmat})
    return in_maps, lengths


def _host_gold(logits, transition, labels, predict_mask):
    T = np.asarray(transition, np.float64)
    lab = np.asarray(labels, np.int64)
    maskf = np.asarray(predict_mask, np.float64)
    logits64 = np.asarray(logits, np.float64)
    start, end = L - 2, L - 1
    unary = np.take_along_axis(logits64, lab[:, :, None], axis=2)[..., 0] * maskf
    labels_ext = np.concatenate(
        [np.full((B, 1), start), lab, np.full((B, 1), end)], 1)
    mask_ext = np.concatenate([np.ones((B, 1)), maskf, np.ones((B, 1))], 1)
    labels_m = np.where(mask_ext > 0, labels_ext, end).astype(np.int64)
    trn_scr = T[labels_m[:, 1:], labels_m[:, :-1]]
    mask2 = np.concatenate([np.ones((B, 1)), maskf], 1)
    return unary.sum(1) + (trn_scr * mask2).sum(1)


def _fallback_numpy(logits, transition, labels, predict_mask):
    """Pure-host reference implementation (only for unsupported inputs)."""
    logits = np.asarray(logits, np.float64)
    T = np.asarray(transition, np.float64)
    mask = np.asarray(predict_mask)
    Bn, Sn, n = logits.shape
    Ln_ = T.shape[0]
    start, end = Ln_ - 2, Ln_ - 1
    pads = np.full((Bn, Sn, 2), NEG)
    logits_p = np.concatenate([logits, pads], 2)
    alpha = np.full((Bn, Ln_), -100.0)
    alpha[:, start] = 0.0
    for t in range(Sn):
        mat = logits_p[:, t, :, None] + alpha[:, None, :] + T[None]
        m = mat.max(2, keepdims=True)
        a_n = (m[..., 0] + np.log(np.exp(mat - m).sum(2)))
        alpha = np.where(mask[:, t:t + 1] > 0, a_n, alpha)
    mm = (alpha + T[end][None]).max(1, keepdims=True)
    norm = mm[:, 0] + np.log(np.exp(alpha + T[end][None] - mm).sum(1))
    gold = _host_gold(logits, T, labels, mask)
    return (norm - gold).astype(np.float32)


def run_device(in_maps, trace=False, **kw):
    _, _, _, bass_utils, _ = _import_bass()
    nc = build_program()
    return bass_utils.run_bass_kernel_spmd(
        nc, in_maps, core_ids=list(range(NCORES)), trace=trace, **kw)


def _norm_from_outs(outs_list, transition, lengths):
    """ln Z + C0*len per sequence from the per-core [NS, K*BL] outputs."""
    E, _, _, _, _ = _transition_mats(transition)
    o = np.stack([np.asarray(o, np.float64).reshape(NS, K, BL)
                  for o in outs_list], -1)                  # [NS, K, BL, NC]
    alphas = o[0:N].transpose(1, 0, 3, 2).reshape(K, N, B)
    omegas = o[N:NS].transpose(1, 0, 3, 2).reshape(K, N, B)
    lams = np.einsum("ij,kjb->kib", E.T, omegas)            # lam_i = E^T omega_i
    j_of = (np.asarray(lengths, np.int64) - 1) // CL        # 0-based chunk
    lnZ = np.zeros(B)
    ln_scale = np.zeros(B)
    x = alphas[0].copy()
    for i in range(1, K):
        lam = lams[i]
        dot = (lam * x).sum(0)
        fin = j_of == i
        cont = j_of > i
        lnZ[fin] = np.log(dot[fin]) + ln_scale[fin]
        ln_scale[cont] += np.log(dot[cont]) - np.log(lam.sum(0)[cont])
        x = alphas[i]
    return lnZ + C0 * lengths


def kernel(logits, transition, labels, predict_mask):
    logits = np.asarray(logits)
    transition = np.asarray(transition)
    labels = np.asarray(labels)
    predict_mask = np.asarray(predict_mask)
    assert logits.shape == (B, S, N) and transition.shape == (L, L)

    try:
        in_maps, lengths = _host_prep(logits, transition, predict_mask)
    except ValueError:
        return _fallback_numpy(logits, transition, labels, predict_mask)

    res = run_device(in_maps)
    norm = _norm_from_outs(
        [res.results[c]["outs"] for c in range(NCORES)], transition, lengths)
    gold = _host_gold(logits, transition, labels, predict_mask)
    return (norm - gold).astype(np.float32)


# revision 12
# speedup vs baseline: 1.8644x; 1.0156x over previous
"""Trainium2 Bass kernel for linear-chain CRF NLL — chunked rank-1 scan.

v2 halved the serial depth with a forward/backward meet-in-the-middle; this
version removes the latency wall entirely. The chunk transfer matrices
P_i = prod_{t in chunk} (D_t E) are numerically rank-1 (E = exp(T) with
T ~ U[-0.1,0.1] is a small perturbation of the all-ones matrix, so a
64-step product contracts off-rank components by ~1e-50; validated to f32
roundoff in _sim_v3.py). Hence the time axis splits into K independent
chunks; the device only computes, per chunk and per sequence:

  alpha_i = forward pass over chunk i from all-ones state
  omega_i = backward pass over chunk i from all-ones state
            (with the masked-tail / f-handoff emission machinery)

and the host composes Z per sequence with rank-1 cross-approximation dots
  lam_i = E^T omega_i
  x = alpha_1;  for i = 2..j-1:  x = alpha_i * (lam_i.x)/sum(lam_i)
  Z = lam_j . x      (j = chunk containing len; log-scaled, f64)

Each chunk's fwd+bwd passes stack into one 128-partition chain (state
[u; c], block-diag weights [[E,0],[0,E^T]] — same stationary matrix for
everything). K=32 chunks -> 32 chains of 32 rounds, packed G=8 chains per
instruction group: one matmul [128 x 128 x 512] + one elementwise multiply
per group per wave, 4 groups rotating so PE and DVE stay saturated (big
instructions amortize the ~125ns fixed LDWEIGHTS / PSUM-access costs).
The wall-clock is engine-throughput-bound (~99us measured), not chain-
latency bound (v1: 1024 serial rounds x ~650ns = 662us; v2: 513 x ~560ns
= 289us). GpSimd cannot read PSUM on TRN2 and the Scalar-copy bounce is
slower than the DVE's direct PSUM multiply, so all multiplies go to DVE.

All boundary conditions ride on host-fabricated emissions (kappa trick,
see kernel_v2); bf16 state/weights/emissions, fp32 PSUM.
"""

import os
import sys

import numpy as np

S = 1024           # sequence length
N = 64             # n_labels
L = 66             # n_labels + 2 (START, END)
B = 512            # batch
NCORES = 8
BL = B // NCORES   # 64 sequences per core
NS = 2 * N         # stacked state rows [fwd; bwd]
K = int(os.environ.get("V3_K", "32"))   # time chunks (= independent chains)
CL = S // K        # rounds per chain
G = int(os.environ.get("V3_G", "8"))    # chains per instruction group
NG = K // G        # instruction groups
GW = G * BL        # moving columns per grouped instruction
C0 = 4.66          # emission centering constant (~log(64*e^0.5))
NEG = -1000.0
MINLEN = S // 2    # guaranteed minimum sequence length
DMAW = 8           # waves per emission DMA chunk
# mul engine per group: 'v' = DVE (vector), 'g' = GpSimd (pool)
ENGS = os.environ.get("V3_ENGS", "v" * NG)

_BASS_PATHS = (
    "/opt/trn_rl_repo",
    os.path.expanduser("~/.axon_site/_ro/trn_rl_repo"),
)


def _import_bass():
    try:
        import concourse.bass  # noqa: F401
    except ImportError:
        for p in _BASS_PATHS:
            if os.path.isdir(p) and p not in sys.path:
                sys.path.insert(0, p)
    import concourse.bass as bass
    import concourse.bacc as bacc
    import concourse.mybir as mybir
    import concourse.tile as tile
    from concourse import bass_utils
    return bass, mybir, tile, bass_utils, bacc


def _bf16():
    _, mybir, _, _, _ = _import_bass()
    return mybir.dt.np(mybir.dt.bfloat16)


_PROGRAM_CACHE = {}


def build_program():
    """Build the (input-independent) Bass program; returns nc."""
    key = ("nc", ENGS)
    if key in _PROGRAM_CACHE:
        return _PROGRAM_CACHE[key]
    bass, mybir, tile, _, bacc = _import_bass()
    from contextlib import ExitStack

    f32 = mybir.dt.float32
    bf16 = mybir.dt.bfloat16

    nc = bacc.Bacc("TRN2", target_bir_lowering=False, debug=False,
                   enable_asserts=False)
    el = nc.dram_tensor("el", [NS, CL, K * BL], bf16, kind="ExternalInput").ap()
    wmat = nc.dram_tensor("wmat", [NS, NS], bf16, kind="ExternalInput").ap()
    outs = nc.dram_tensor("outs", [NS, K * BL], bf16, kind="ExternalOutput").ap()

    with tile.TileContext(nc) as tc, ExitStack() as ctx:
        consts = ctx.enter_context(tc.tile_pool(name="consts", bufs=1))
        els = ctx.enter_context(tc.tile_pool(name="els", bufs=1))
        vs = ctx.enter_context(tc.tile_pool(name="vs", bufs=3))
        opool = ctx.enter_context(tc.tile_pool(name="opool", bufs=1))
        qpool = ctx.enter_context(tc.tile_pool(name="qpool", bufs=2, space="PSUM"))

        # small leading chunks so wave 0 can start before the bulk arrives;
        # chunk 0 is issued BEFORE the weights DMA: the first multiply gates
        # on it, while the first matmul (needing weights) has ~2us of slack
        # behind the engine-boot sequence anyway.
        sizes = [1, 3, 4] + [DMAW] * ((CL - 8) // DMAW)
        assert sum(sizes) == CL
        el_t = []       # (tile, first_wave, n_waves) spans
        wsb = consts.tile([NS, NS], bf16)
        lo = 0
        for c, n in enumerate(sizes):
            t = els.tile([NS, n, K * BL], bf16, tag=f"el{c}")
            nc.sync.dma_start(out=t, in_=el[:, lo:lo + n, :])
            el_t.append((t, lo, n))
            lo += n
            if c == 0:
                nc.sync.dma_start(out=wsb, in_=wmat)

        mul_eng = [nc.vector if e == "v" else nc.gpsimd for e in ENGS]
        s = []
        for g in range(NG):
            t = vs.tile([NS, GW], bf16, tag=f"s{g}")
            nc.vector.memset(t, 1.0)
            s.append(t)
        chunk_of = []
        for t, lo, n in el_t:
            chunk_of += [(t, lo)] * n
        for tau in range(CL):
            et, lo = chunk_of[tau]
            for g in range(NG):
                q = qpool.tile([NS, GW], f32, tag=f"q{g}")
                nc.tensor.matmul(q, wsb, s[g], start=True, stop=True)
                s_new = vs.tile([NS, GW], bf16, tag=f"s{g}")
                mul_eng[g].tensor_mul(
                    s_new, et[:, tau - lo, g * GW:(g + 1) * GW], q)
                s[g] = s_new
        # DMA each group's final bf16 state out directly (no f32 conversion
        # copy); the host composes in f64 anyway. Each group's DMA can issue
        # as soon as that group's last multiply lands.
        for g in range(NG):
            nc.sync.dma_start(out=outs[:, g * GW:(g + 1) * GW], in_=s[g])

    nc.compile()
    _PROGRAM_CACHE[key] = nc
    return nc


def _transition_mats(transition):
    """E, f, st, g_f, g_b in float64 from the [66,66] transition matrix."""
    T = np.asarray(transition, np.float64)
    E = np.exp(T[0:N, 0:N])          # E[to, frm]
    f = np.exp(T[L - 1, 0:N])        # into END
    st = np.exp(T[0:N, L - 2])       # from START
    g_f = E.sum(axis=1)              # E @ 1   (forward kappa correction)
    g_b = E.sum(axis=0)              # E^T @ 1 (backward kappa correction)
    return E, f, st, g_f, g_b


def make_wmat(transition):
    """Stationary block-diag matrix in lhsT layout [NS, NS]:
    out rows 0:64 = E u  ->  lhsT[k, i] = E[i, k];
    out rows 64:128 = E^T c -> lhsT[64+k, 64+i] = E[k, i]."""
    E, _, _, _, _ = _transition_mats(transition)
    wmat = np.zeros((NS, NS), np.float64)
    wmat[0:N, 0:N] = E.T
    wmat[N:NS, N:NS] = E
    return wmat.astype(_bf16())


def _fabricate_el(logits, transition, lengths):
    """Emission stream [NS, CL, K, B] float32 (pre-bf16, pre-core-slice)."""
    E, f, st, g_f, g_b = _transition_mats(transition)
    base = np.exp(np.asarray(logits, np.float32) - np.float32(C0))  # [B,S,N]
    baseT = base.transpose(1, 2, 0)                                 # [S,N,B]

    elf = baseT.copy()
    elf[0] = elf[0] * (st / g_f)[:, None].astype(np.float32)

    t_idx = np.arange(1, S + 1)
    hand = (t_idx[:, None] == lengths[None, :])                     # [S,B]
    mskd = (t_idx[:, None] > lengths[None, :])
    elb = baseT * np.where(hand[:, None, :], (f / g_b)[:, None],
                           1.0).astype(np.float32)
    elb = np.where(mskd[:, None, :],
                   (1.0 / g_b)[:, None].astype(np.float32), elb)

    elf_r = elf.reshape(K, CL, N, B)            # [k, tau] = t: k*CL+tau+1
    elb_r = elb.reshape(K, CL, N, B)[:, ::-1]   # [k, tau] = t: (k+1)*CL-tau
    el_all = np.concatenate([elf_r, elb_r], axis=2)  # [K, CL, NS, B]
    return el_all.transpose(2, 1, 0, 3)              # [NS, CL, K, B]


def _host_prep(logits, transition, predict_mask):
    """Returns (in_maps, lengths). Raises ValueError if inputs unsupported."""
    lengths = np.asarray(predict_mask, np.int64).sum(1)
    prefix = (np.asarray(predict_mask, np.int64)
              == (np.arange(S)[None, :] < lengths[:, None])).all()
    if not prefix or lengths.min() < MINLEN:
        raise ValueError("mask is not a contiguous prefix with len >= S/2")

    el_all = _fabricate_el(logits, transition, lengths)  # [NS, CL, K, B]
    wmat = make_wmat(transition)
    bf = _bf16()

    in_maps = []
    for c in range(NCORES):
        el_core = np.ascontiguousarray(
            el_all[:, :, :, c * BL:(c + 1) * BL]).reshape(NS, CL, K * BL)
        in_maps.append({"el": el_core.astype(bf), "wmat": wmat})
    return in_maps, lengths


def _host_gold(logits, transition, labels, predict_mask):
    T = np.asarray(transition, np.float64)
    lab = np.asarray(labels, np.int64)
    maskf = np.asarray(predict_mask, np.float64)
    logits64 = np.asarray(logits, np.float64)
    start, end = L - 2, L - 1
    unary = np.take_along_axis(logits64, lab[:, :, None], axis=2)[..., 0] * maskf
    labels_ext = np.concatenate(
        [np.full((B, 1), start), lab, np.full((B, 1), end)], 1)
    mask_ext = np.concatenate([np.ones((B, 1)), maskf, np.ones((B, 1))], 1)
    labels_m = np.where(mask_ext > 0, labels_ext, end).astype(np.int64)
    trn_scr = T[labels_m[:, 1:], labels_m[:, :-1]]
    mask2 = np.concatenate([np.ones((B, 1)), maskf], 1)
    return unary.sum(1) + (trn_scr * mask2).sum(1)


def _fallback_numpy(logits, transition, labels, predict_mask):
    """Pure-host reference implementation (only for unsupported inputs)."""
    logits = np.asarray(logits, np.float64)
    T = np.asarray(transition, np.float64)
    mask = np.asarray(predict_mask)
    Bn, Sn, n = logits.shape
    Ln_ = T.shape[0]
    start, end = Ln_ - 2, Ln_ - 1
    pads = np.full((Bn, Sn, 2), NEG)
    logits_p = np.concatenate([logits, pads], 2)
    alpha = np.full((Bn, Ln_), -100.0)
    alpha[:, start] = 0.0
    for t in range(Sn):
        mat = logits_p[:, t, :, None] + alpha[:, None, :] + T[None]
        m = mat.max(2, keepdims=True)
        a_n = (m[..., 0] + np.log(np.exp(mat - m).sum(2)))
        alpha = np.where(mask[:, t:t + 1] > 0, a_n, alpha)
    mm = (alpha + T[end][None]).max(1, keepdims=True)
    norm = mm[:, 0] + np.log(np.exp(alpha + T[end][None] - mm).sum(1))
    gold = _host_gold(logits, T, labels, mask)
    return (norm - gold).astype(np.float32)


def run_device(in_maps, trace=False, **kw):
    _, _, _, bass_utils, _ = _import_bass()
    nc = build_program()
    return bass_utils.run_bass_kernel_spmd(
        nc, in_maps, core_ids=list(range(NCORES)), trace=trace, **kw)


def _norm_from_outs(outs_list, transition, lengths):
    """ln Z + C0*len per sequence from the per-core [NS, K*BL] outputs."""
    E, _, _, _, _ = _transition_mats(transition)
    o = np.stack([np.asarray(o, np.float64).reshape(NS, K, BL)
                  for o in outs_list], -1)                  # [NS, K, BL, NC]
    alphas = o[0:N].transpose(1, 0, 3, 2).reshape(K, N, B)
    omegas = o[N:NS].transpose(1, 0, 3, 2).reshape(K, N, B)
    lams = np.einsum("ij,kjb->kib", E.T, omegas)            # lam_i = E^T omega_i
    j_of = (np.asarray(lengths, np.int64) - 1) // CL        # 0-based chunk
    lnZ = np.zeros(B)
    ln_scale = np.zeros(B)
    x = alphas[0].copy()
    for i in range(1, K):
        lam = lams[i]
        dot = (lam * x).sum(0)
        fin = j_of == i
        cont = j_of > i
        lnZ[fin] = np.log(dot[fin]) + ln_scale[fin]
        ln_scale[cont] += np.log(dot[cont]) - np.log(lam.sum(0)[cont])
        x = alphas[i]
    return lnZ + C0 * lengths


def kernel(logits, transition, labels, predict_mask):
    logits = np.asarray(logits)
    transition = np.asarray(transition)
    labels = np.asarray(labels)
    predict_mask = np.asarray(predict_mask)
    assert logits.shape == (B, S, N) and transition.shape == (L, L)

    try:
        in_maps, lengths = _host_prep(logits, transition, predict_mask)
    except ValueError:
        return _fallback_numpy(logits, transition, labels, predict_mask)

    res = run_device(in_maps)
    norm = _norm_from_outs(
        [res.results[c]["outs"] for c in range(NCORES)], transition, lengths)
    gold = _host_gold(logits, transition, labels, predict_mask)
    return (norm - gold).astype(np.float32)
